# revision 7
# baseline (speedup 1.0000x reference)
"""Trainium2 Bass kernel for Longformer self-attention (B=2, S=4096, D=768, H=12, HD=64, W=256, G=32).

Sharding: 8 cores = 2 batches x 4 head-groups (3 heads each). Each core computes its
batch's projections restricted to its 192 output channels, runs banded + global
attention for its 3 heads, and writes a disjoint [4096, 192] slice of the output.
No collectives needed.

On-chip layout (per core):
  qT/kT/kgT  : transposed projections, head-dim on partitions ([128]=heads01, [64]=head2)
  v/vg       : natural layout [128-part seq tiles, per-head 64 cols + ones column]
  band scores: computed transposed ([128 keys, 640 query span] per key block), so
               softmax denominator comes free via the ones column in the PV matmul.
Matmul inputs are bf16 (hidden_states pre-cast on host), accumulation fp32,
softmax/normalization fp32.
"""
import numpy as np
import ml_dtypes

import concourse.bass as bass
import concourse.mybir as mybir
import concourse.tile as tile
from concourse import bacc
from concourse.bass_utils import run_bass_kernel_spmd

B, S, D, H, HD = 2, 4096, 768, 12, 64
W = 256
G = 32
SCALE = 1.0 / np.float32(np.sqrt(HD))
NEG = -60.0
KB = 128          # key block
NKB = S // KB     # 32
QSB = 512         # query superblock
NQSB = S // QSB   # 8
NKT = D // 128    # 6 contraction tiles
NNT = S // 512    # 8 seq chunks for projections

BF = mybir.dt.bfloat16
F32 = mybir.dt.float32
AF = mybir.ActivationFunctionType
bf16 = ml_dtypes.bfloat16

_cache = {}


def _span(kb):
    """Canonical 640-wide query span of key block kb, clipped to [0, S)."""
    k0 = KB * kb
    qlo, qhi = max(0, k0 - 2 * KB), min(S, k0 + 3 * KB)
    llo = qlo - (k0 - 2 * KB)
    lhi = qhi - (k0 - 2 * KB)
    return qlo, qhi, llo, lhi


def _build():
    nc = bacc.Bacc(None, target_bir_lowering=False)

    hsb = nc.declare_dram_parameter("hsb", [S, D], BF, isOutput=False)
    w_q = nc.declare_dram_parameter("w_q", [128, NKT, 192], BF, isOutput=False)
    w_k = nc.declare_dram_parameter("w_k", [128, NKT, 192], BF, isOutput=False)
    w_kg = nc.declare_dram_parameter("w_kg", [128, NKT, 192], BF, isOutput=False)
    w_qg = nc.declare_dram_parameter("w_qg", [128, NKT, 192], BF, isOutput=False)
    w_vvg = nc.declare_dram_parameter("w_vvg", [128, NKT, 384], BF, isOutput=False)
    bvvg_d = nc.declare_dram_parameter("bvvg", [1, 384], BF, isOutput=False)
    bias_d = nc.declare_dram_parameter("bias_t", [128, 8], F32, isOutput=False)
    masks_d = nc.declare_dram_parameter("masks", [128, 256], F32, isOutput=False)
    id65_d = nc.declare_dram_parameter("id65", [65, 65], F32, isOutput=False)
    id96_d = nc.declare_dram_parameter("id96", [96, 96], BF, isOutput=False)
    out_d = nc.declare_dram_parameter("out", [S, 192], F32, isOutput=True)

    with tile.TileContext(nc) as tc:
        with tc.tile_pool(name="persist", bufs=1) as pp:
            wq_t = pp.tile([128, NKT, 192], BF)
            wk_t = pp.tile([128, NKT, 192], BF)
            wkg_t = pp.tile([128, NKT, 192], BF)
            wqg_t = pp.tile([128, NKT, 192], BF)
            wvvg_t = pp.tile([128, NKT, 384], BF)
            bvvg_t = pp.tile([1, 384], BF)
            bias_t = pp.tile([128, 8], F32)
            masks_t = pp.tile([128, 256], F32)
            id65_t = pp.tile([65, 65], F32)
            id96_t = pp.tile([96, 96], BF)
            ones_t = pp.tile([1, 128], BF)

            nc.sync.dma_start(wq_t[:], w_q[:])
            nc.sync.dma_start(wk_t[:], w_k[:])
            nc.sync.dma_start(wkg_t[:], w_kg[:])
            nc.sync.dma_start(wqg_t[:], w_qg[:])
            nc.sync.dma_start(wvvg_t[:], w_vvg[:])
            nc.sync.dma_start(bvvg_t[:], bvvg_d[:])
            nc.sync.dma_start(bias_t[:], bias_d[:])
            nc.sync.dma_start(masks_t[:], masks_d[:])
            nc.sync.dma_start(id65_t[:], id65_d[:])
            nc.sync.dma_start(id96_t[:], id96_d[:])
            nc.vector.memset(ones_t[:], 1.0)

            qT01 = pp.tile([128, S], BF)
            qT2 = pp.tile([64, S], BF)
            kT01 = pp.tile([128, S], BF)
            kT2 = pp.tile([64, S], BF)
            kgT01 = pp.tile([128, S], BF)
            kgT2 = pp.tile([64, S], BF)
            v_nat = pp.tile([128, NKB, 3, 65], BF)
            vg_nat = pp.tile([128, NKB, 3, 65], BF)
            qgT01 = pp.tile([128, G], BF)
            qgT2 = pp.tile([64, G], BF)
            exp_sg = pp.tile([G, 3, S], BF)
            probs_g = pp.tile([96, S], BF)
            pb_gT = pp.tile([128, NKB, 96], BF)
            g_sb = pp.tile([G, 192], F32)
            staging = pp.tile([128, NKB, 192], F32)

            nc.vector.memset(v_nat[:, :, :, 64:65], 1.0)
            nc.vector.memset(vg_nat[:, :, :, 64:65], 1.0)

            def sl01(t01, t2, h):
                return t01[64 * h:64 * h + 64] if h < 2 else t2[:]

            # ---------------- Phase A: projections (streamed hsT chunks) --------------
            with (
                tc.tile_pool(name="hst", bufs=3) as hstp,
                tc.tile_pool(name="apsum", bufs=6, space="PSUM") as apsum,
            ):
                tproj = [
                    (wq_t, 0, 1, qT01, qT2),
                    (wk_t, 2, 3, kT01, kT2),
                    (wkg_t, 4, 5, kgT01, kgT2),
                ]
                for nt in range(NNT):
                    c0 = 512 * nt
                    hst = hstp.tile([128, NKT, 512], BF)
                    for kt in range(NKT):
                        nc.sync.dma_start(
                            out=hst[:, kt, :],
                            in_=hsb[c0:c0 + 512, 128 * kt:128 * kt + 128],
                            transpose=True,
                        )
                    for pi, (wt, c01, c2, d01, d2) in enumerate(tproj):
                        ps = apsum.tile([128, 512], F32, tag="pp")
                        for kt in range(NKT):
                            nc.tensor.matmul(ps[:], wt[:, kt, 0:128], hst[:, kt, :],
                                             start=(kt == 0), stop=(kt == NKT - 1))
                        nc.vector.tensor_scalar_add(
                            d01[:, c0:c0 + 512], ps[:], bias_t[:, c01:c01 + 1])
                        ps2 = apsum.tile([64, 512], F32, tag="pp")
                        for kt in range(NKT):
                            nc.tensor.matmul(ps2[:], wt[:, kt, 128:192], hst[:, kt, :],
                                             start=(kt == 0), stop=(kt == NKT - 1))
                        nc.scalar.activation(
                            d2[:, c0:c0 + 512], ps2[:], AF.Identity,
                            bias=bias_t[0:64, c2:c2 + 1], scale=1.0)
                    for s4 in range(4):
                        sb = 4 * nt + s4
                        psv = apsum.tile([128, 384], F32, tag="pp")
                        for kt in range(NKT):
                            nc.tensor.matmul(psv[:], hst[:, kt, 128 * s4:128 * s4 + 128],
                                             wvvg_t[:, kt, :],
                                             start=(kt == 0), stop=False)
                        nc.tensor.matmul(psv[:], ones_t[:, 0:128], bvvg_t[:],
                                         start=False, stop=True)
                        nc.vector.tensor_copy(
                            v_nat[:, sb, :, 0:64],
                            psv[:, 0:192].rearrange("p (h e) -> p h e", h=3))
                        nc.scalar.copy(
                            vg_nat[:, sb, :, 0:64],
                            psv[:, 192:384].rearrange("p (h e) -> p h e", h=3))
                    if nt == 0:
                        psq = apsum.tile([128, G], F32, tag="pp")
                        for kt in range(NKT):
                            nc.tensor.matmul(psq[:], wqg_t[:, kt, 0:128], hst[:, kt, 0:G],
                                             start=(kt == 0), stop=(kt == NKT - 1))
                        nc.vector.tensor_scalar_add(qgT01[:], psq[:], bias_t[:, 6:7])
                        psq2 = apsum.tile([64, G], F32, tag="pp")
                        for kt in range(NKT):
                            nc.tensor.matmul(psq2[:], wqg_t[:, kt, 128:192], hst[:, kt, 0:G],
                                             start=(kt == 0), stop=(kt == NKT - 1))
                        nc.vector.tensor_scalar_add(qgT2[:], psq2[:], bias_t[0:64, 7:8])

            # ------------- Phase B: global-key scores exp(sg) [G, 3, S] ----------------
            with tc.tile_pool(name="bpsum", bufs=2, space="PSUM") as bpsum:
                for c8 in range(NNT):
                    c0 = 512 * c8
                    ps = bpsum.tile([G, 3, 512], F32, tag="sg")
                    for h in range(3):
                        nc.tensor.matmul(ps[:, h, :],
                                         sl01(kT01, kT2, h)[:, 0:G],
                                         sl01(qT01, qT2, h)[:, c0:c0 + 512])
                    nc.scalar.activation(exp_sg[:, :, c0:c0 + 512], ps[:], AF.Exp)

            # ------------- Phase C: global-query attention (overwrites rows :G) -------
            with (
                tc.tile_pool(name="cpsum", bufs=2, space="PSUM") as cpsum,
                tc.tile_pool(name="cgps", bufs=3, space="PSUM") as cgps,
            ):
                for c8 in range(NNT):
                    c0 = 512 * c8
                    ps = cpsum.tile([96, 512], F32, tag="qg")
                    for h in range(3):
                        nc.tensor.matmul(ps[32 * h:32 * h + 32, :],
                                         sl01(qgT01, qgT2, h)[:],
                                         sl01(kgT01, kgT2, h)[:, c0:c0 + 512])
                    nc.scalar.activation(probs_g[:, c0:c0 + 512], ps[:], AF.Exp)
                for t in range(NKB):
                    pst = cpsum.tile([128, 96], BF, tag="qg")
                    nc.tensor.transpose(pst[:], probs_g[:, 128 * t:128 * t + 128], id96_t[:])
                    if t % 2 == 0:
                        nc.vector.tensor_copy(pb_gT[:, t, :], pst[:])
                    else:
                        nc.scalar.copy(pb_gT[:, t, :], pst[:])
                for h in range(3):
                    go = cgps.tile([G, 65], F32, tag="go")
                    for t in range(NKB):
                        nc.tensor.matmul(go[:], pb_gT[:, t, 32 * h:32 * h + 32],
                                         vg_nat[:, t, h, :],
                                         start=(t == 0), stop=(t == NKB - 1))
                    rz = cgps.tile([G, 1], F32, tag="rz")
                    nc.vector.reciprocal(rz[:], go[:, 64:65])
                    nc.vector.tensor_scalar_mul(g_sb[:, 64 * h:64 * h + 64],
                                                go[:, 0:64], rz[:])

            # ------------- Phase D: banded attention ----------------------------------
            with (
                tc.tile_pool(name="spsum", bufs=2, space="PSUM") as spsum,
                tc.tile_pool(name="opsum", bufs=2, space="PSUM") as opsum,
                tc.tile_pool(name="tpsum", bufs=2, space="PSUM") as tpsum,
                tc.tile_pool(name="pbt", bufs=12) as pbtp,
                tc.tile_pool(name="osb", bufs=3) as osbp,
                tc.tile_pool(name="rt", bufs=4) as rtp,
            ):
                for h in range(3):
                    qTh = sl01(qT01, qT2, h)
                    kTh = sl01(kT01, kT2, h)
                    pbt = {}

                    def do_kb(kb):
                        k0 = KB * kb
                        qlo, qhi, llo, lhi = _span(kb)
                        ps = spsum.tile([128, 640], F32, tag="sc")
                        for (a, b2) in [(llo, min(lhi, 512)), (max(llo, 512), lhi)]:
                            if a >= b2:
                                continue
                            nc.tensor.matmul(ps[:, a:b2], kTh[:, k0:k0 + KB],
                                             qTh[:, qlo + (a - llo):qlo + (a - llo) + (b2 - a)])
                        if llo == 0:
                            nc.vector.tensor_add(ps[:, 0:KB], ps[:, 0:KB],
                                                 masks_t[:, 0:KB])
                        if lhi == 5 * KB:
                            nc.vector.tensor_add(ps[:, 512:640], ps[:, 512:640],
                                                 masks_t[:, KB:2 * KB])
                        t_ = pbtp.tile([128, 640], BF, tag="pb")
                        nc.scalar.activation(t_[:, llo:lhi], ps[:, llo:lhi], AF.Exp)
                        pbt[kb] = t_

                    for qs in range(NQSB):
                        q0 = QSB * qs
                        new_kbs = range(0, 6) if qs == 0 else range(4 * qs + 2,
                                                                    min(NKB, 4 * qs + 6))
                        for kb in new_kbs:
                            do_kb(kb)
                        po = opsum.tile([65, 512], F32, tag="po")
                        nc.tensor.matmul(po[:], v_nat[0:G, 0, h, :],
                                         exp_sg[:, h, q0:q0 + 512],
                                         start=True, stop=False)
                        kbs = list(range(max(0, 4 * qs - 2), min(NKB, 4 * qs + 6)))
                        for i, kb in enumerate(kbs):
                            k0 = KB * kb
                            qlo, qhi, llo, lhi = _span(kb)
                            a, b2 = max(qlo, q0), min(qhi, q0 + QSB)
                            la = a - (k0 - 2 * KB)
                            nc.tensor.matmul(po[:, a - q0:b2 - q0], v_nat[:, kb, h, :],
                                             pbt[kb][:, la:la + (b2 - a)],
                                             start=False, stop=(i == len(kbs) - 1))
                        osb = osbp.tile([65, 512], F32, tag="ob")
                        if qs % 2 == 0:
                            nc.vector.tensor_copy(osb[:], po[:])
                        else:
                            nc.scalar.copy(osb[:], po[:])
                        for s4 in range(4):
                            sb = 4 * qs + s4
                            pt = tpsum.tile([128, 65], F32, tag="pt")
                            nc.tensor.transpose(pt[:], osb[:, 128 * s4:128 * s4 + 128],
                                                id65_t[:])
                            rt = rtp.tile([128, 1], F32, tag="rt")
                            nc.vector.reciprocal(rt[:], pt[:, 64:65])
                            nc.vector.tensor_scalar_mul(
                                staging[:, sb, 64 * h:64 * h + 64], pt[:, 0:64], rt[:])

            # ------------- Phase E: global-query overwrite + store --------------------
            nc.vector.tensor_copy(staging[0:G, 0, :], g_sb[:])
            out_r = out_d.rearrange("(t p) c -> p t c", p=128)
            for a in range(4):
                nc.sync.dma_start(out_r[:, 8 * a:8 * a + 8, :],
                                  staging[:, 8 * a:8 * a + 8, :])

    nc.compile()
    return nc


def _prep_inputs(inputs):
    hs = np.asarray(inputs["hidden_states"], dtype=np.float32)
    maps = []
    j = np.arange(KB)
    p = np.arange(KB)[:, None]
    m_lo = np.where(j[None, :] >= p, 0.0, NEG).astype(np.float32)
    m_hi = np.where(j[None, :] <= p, 0.0, NEG).astype(np.float32)
    masks = np.concatenate([m_lo, m_hi], axis=1)
    id65 = np.eye(65, dtype=np.float32)
    id96 = np.eye(96, dtype=bf16)

    def wtiles(w):
        # [768, n] -> [128, 6, n] bf16
        n = w.shape[1]
        return np.ascontiguousarray(
            w.reshape(NKT, 128, n).transpose(1, 0, 2)).astype(bf16)

    for c in range(8):
        b, hg = c // 4, c % 4
        cols = slice(192 * hg, 192 * hg + 192)
        Wq = np.asarray(inputs["Wq"], np.float32)[:, cols] * SCALE
        bq = np.asarray(inputs["bq"], np.float32)[cols] * SCALE
        Wqg = np.asarray(inputs["Wqg"], np.float32)[:, cols] * SCALE
        bqg = np.asarray(inputs["bqg"], np.float32)[cols] * SCALE
        Wk = np.asarray(inputs["Wk"], np.float32)[:, cols]
        bk = np.asarray(inputs["bk"], np.float32)[cols]
        Wkg = np.asarray(inputs["Wkg"], np.float32)[:, cols]
        bkg = np.asarray(inputs["bkg"], np.float32)[cols]
        Wv = np.asarray(inputs["Wv"], np.float32)[:, cols]
        bv = np.asarray(inputs["bv"], np.float32)[cols]
        Wvg = np.asarray(inputs["Wvg"], np.float32)[:, cols]
        bvg = np.asarray(inputs["bvg"], np.float32)[cols]

        # column order in kernel: 0 q01, 1 q2, 2 k01, 3 k2, 4 kg01, 5 kg2, 6 qg01, 7 qg2
        bias_t2 = np.zeros((128, 8), np.float32)
        bias_t2[:, 0], bias_t2[0:64, 1] = bq[0:128], bq[128:192]
        bias_t2[:, 2], bias_t2[0:64, 3] = bk[0:128], bk[128:192]
        bias_t2[:, 4], bias_t2[0:64, 5] = bkg[0:128], bkg[128:192]
        bias_t2[:, 6], bias_t2[0:64, 7] = bqg[0:128], bqg[128:192]

        maps.append({
            "hsb": hs[b].astype(bf16),
            "w_q": wtiles(Wq),
            "w_k": wtiles(Wk),
            "w_kg": wtiles(Wkg),
            "w_qg": wtiles(Wqg),
            "w_vvg": wtiles(np.concatenate([Wv, Wvg], axis=1)),
            "bvvg": np.concatenate([bv, bvg])[None, :].astype(bf16),
            "bias_t": bias_t2,
            "masks": masks,
            "id65": id65,
            "id96": id96,
        })
    return maps


def kernel(**inputs):
    g = int(np.asarray(inputs["num_global"]))
    assert g == G, f"kernel compiled for num_global=32, got {g}"
    if "nc" not in _cache:
        _cache["nc"] = _build()
    nc = _cache["nc"]
    in_maps = _prep_inputs(inputs)
    res = run_bass_kernel_spmd(nc, in_maps, list(range(8)))
    out = np.zeros((B, S, D), np.float32)
    for c in range(8):
        out[c // 4, :, 192 * (c % 4):192 * (c % 4) + 192] = res.results[c]["out"]
    return out
